# revision 32
# baseline (speedup 1.0000x reference)
"""Trainium2 Bass kernel for causal multi-head attention (dense transformer block).

Problem: x[2,2048,1024] -> qkv proj -> 16-head causal attention (scale 1/sqrt(1024))
         -> out proj.  8 NeuronCores.

Sharding: core c handles batch b=c//4 and head-group r=c%4 (heads 4r..4r+3).
  - qkv weights column-sharded by head group (q/k/v slices of 256 cols each)
  - Q/K projection runs in fp8e4 DoubleRow (dm-chunk pairs as the second
    contraction plane, 0.5 cycles/row); Q/K are only consumed by the fp8
    score matmul, so the extra quantization is cheap in accuracy.
  - S^T[k,q] = K^T (stationary) x Q^T (moving), fp8e4 DoubleRow with a
    zeroed second plane (head_dim 64 lives on 64 partitions; plane 1 of K^T
    is zeros so it contracts to 0).  Scores for two k-tiles share one PSUM
    tile so one exp() activation covers both (halves the Act-engine's fixed
    per-instruction SBUF-access overhead).
  - P = exp(S/32) (bf16) with causal masking; attention output accumulated
    token-major: acc[q, 65] += P_blk^T x [V|1] per k-tile (PSUM partition dim
    stays 128; the softmax denominator is a free column, normalized by a
    per-partition reciprocal multiply on the DVE).
  - The out-proj follows an AllGather of O^T (bf16, groups of 4 cores).
  - 4-pass software pipeline: QKV projection for token-block p+1 and deferred
    out-proj work are spliced into pass p's attention job stream so the PE
    stays busy while the Activation engine works through exp().

kernel(**inputs) takes the FULL fp32 inputs and returns the FULL output.
"""

import sys

sys.path.insert(0, "/opt/trn_rl_repo")

import numpy as np

import concourse.bass as bass
import concourse.bacc as bacc
import concourse.mybir as mybir
import concourse.tile as tile
from concourse.bass import ds, ts
from concourse.bass_utils import run_bass_kernel_spmd
from concourse.masks import make_upper_triangular

F32 = mybir.dt.float32
BF16 = mybir.dt.bfloat16
FP8 = mybir.dt.float8e4

# ---------------------------------------------------------------- dims
BS, L, DM, H = 2, 2048, 1024, 16
HD = 64                      # head dim
NCORES = 8
GRP = 4                      # cores per batch group (head-parallel)
HLOC = H // GRP              # heads per core = 4
FLOC = HLOC * HD             # local features = 256
SCALE = 1.0 / float(np.sqrt(DM))
REPLICA_GROUPS = [[0, 1, 2, 3], [4, 5, 6, 7]]


class Cfg:
    def __init__(self, L=L, DM=DM, hloc=HLOC, hd=HD, npass=4, nwarm=32,
                 use_fp8=True, fp8_proj=True):
        self.L, self.DM, self.HLOC, self.HD, self.NPASS = L, DM, hloc, hd, npass
        self.FLOC = hloc * hd
        self.NT = L // 128           # 128-token tiles
        self.NB = L // 512           # 512-token blocks
        self.NDM = DM // 128         # dmodel chunks
        self.PW = L // npass         # pass width (q columns per pass)
        self.NFT = self.FLOC // 128  # feature tiles for O^T (2)
        self.NWARM = nwarm
        self.USE_FP8 = use_fp8
        self.FP8_PROJ = fp8_proj and use_fp8
        self.scale = 1.0 / float(np.sqrt(DM))
        assert self.PW == 512 and self.FLOC % 128 == 0


def build_body(nc, cfg, x, wqkv, bq, bk, bv, wo, bo, out, groups, dbg_hook=None):
    """Emit the per-core program (Tile framework)."""
    NT, NB, NDM, NFT = cfg.NT, cfg.NB, cfg.NDM, cfg.NFT
    HLOCc, HDc, FLOCc = cfg.HLOC, cfg.HD, cfg.FLOC
    Lc, DMc = cfg.L, cfg.DM
    NPASS = cfg.NPASS
    QKDT = FP8 if cfg.USE_FP8 else BF16
    tc = nc.tc
    DR = mybir.MatmulPerfMode.DoubleRow

    with tc.tile_pool(name="const", bufs=1) as constp, \
         tc.tile_pool(name="persist", bufs=1) as pp, \
         tc.tile_pool(name="stage", bufs=2) as sp, \
         tc.tile_pool(name="stage4", bufs=4) as sp4, \
         tc.tile_pool(name="pbuf", bufs=13) as pbp, \
         tc.tile_pool(name="nrm", bufs=4) as nrm, \
         tc.tile_pool(name="otm", bufs=2) as otmp, \
         tc.tile_pool(name="of", bufs=2) as ofp, \
         tc.tile_pool(name="osb", bufs=2) as osbp, \
         tc.tile_pool(name="dram", bufs=2, space="DRAM") as dramp:
        # ---------------- persistent SBUF tensors
        xT = pp.tile([128, NDM, Lc], BF16)                 # x^T  (dm-major)
        wqkvb = pp.tile([128, NDM, 3 * FLOCc], BF16)       # [wq|wk|wv] packed
        wqb = wqkvb[:, :, 0:FLOCc]
        wkb = wqkvb[:, :, FLOCc : 2 * FLOCc]
        wvb = wqkvb[:, :, 2 * FLOCc : 3 * FLOCc]
        wob = pp.tile([128, NDM, FLOCc], BF16)
        if cfg.FP8_PROJ:
            x8T = pp.tile([128, NDM, Lc], FP8)             # fp8 copy of x^T
            wqk8 = pp.tile([128, NDM, 2 * FLOCc], FP8)     # fp8 [wq|wk]
        # Q^T/K^T feature-major with a DoubleRow plane dim: chunk hf holds
        # heads 2hf,2hf+1 (partition 64*(h%2)+hd); plane 0 = data, plane 1 =
        # zeros so the fp8 DoubleRow matmul contracts (K,Q) + (0,0).
        QT8 = pp.tile([128, 2, 2, Lc], QKDT)
        KT8 = pp.tile([128, 2, 2, Lc], QKDT)
        Vb = pp.tile([128, NT, HLOCc * (HDc + 1)], BF16)   # [V | ones] per token tile
        OTs = pp.tile([128, NFT, Lc], BF16)                # attention out^T (feature-major)

        # ---------------- single PSUM pool for the whole kernel
        # banks: stile [128,1024] x2 = 4, acc [128,512] x2 = 2,
        #        work [128,512] x2 = 2  -> 8 banks
        # NOTE: matmul start=True marks the enclosing 2KB zero-region pending,
        # so each accumulation chain owns a full bank (acc padded to 512 f32)
        # and runs its matmuls consecutively.
        psum_cm = tc.tile_pool(name="psum", bufs=2, space="PSUM")
        psum = psum_cm.__enter__()

        # PE warmup: junk matmuls so the p-state ramp happens on the DMA-bound
        # front, not on the first real matmuls.
        wsrc_t = pp.tile([128, 512], BF16, name="wsrc_t")
        nc.vector.memset(wsrc_t, 0.25)
        wps = psum.tile([128, 512], F32, tag="work", name="wps")
        for r in range(cfg.NWARM):
            nc.tensor.matmul(wps, wsrc_t[:, 0:128], wsrc_t,
                             start=(r == 0), stop=(r == cfg.NWARM - 1))
        wout_t = pp.tile([128, 512], F32, name="wout_t")
        nc.vector.tensor_copy(wout_t, wps)
        # preload the Exp activation table during the front
        wexp_t = pp.tile([1, 1], BF16, name="wexp_t")
        nc.scalar.activation(wexp_t, wsrc_t[0:1, 0:1],
                             mybir.ActivationFunctionType.Exp)

        # ---------------- constants
        trimask = constp.tile([128, 128], BF16)
        ones_r = constp.tile([1, 128], BF16)
        bq_f = constp.tile([128, 2], F32)
        bk_f = constp.tile([128, 2], F32)
        bvb = constp.tile([1, FLOCc], BF16)
        bob = constp.tile([1, FLOCc], BF16)

        def emit_consts():
            make_upper_triangular(nc, trimask, val=1.0, diag=True)
            nc.vector.memset(ones_r, 1.0)
            nc.sync.dma_start(bq_f, bq.rearrange("(f p) -> p f", p=128))
            nc.sync.dma_start(bk_f, bk.rearrange("(f p) -> p f", p=128))
            bv_st = constp.tile([1, 2 * FLOCc], F32, name="bv_st")
            nc.sync.dma_start(bv_st[:, 0:FLOCc], bv.rearrange("(a b) -> a b", a=1))
            nc.sync.dma_start(bv_st[:, FLOCc : 2 * FLOCc], bo.rearrange("(a b) -> a b", a=1))
            nc.vector.tensor_copy(bvb, bv_st[:, 0:FLOCc])
            nc.vector.tensor_copy(bob, bv_st[:, FLOCc : 2 * FLOCc])
            nc.vector.memset(
                Vb.rearrange("p t (h u) -> p t h u", u=HDc + 1)[:, :, :, HDc : HDc + 1], 1.0
            )

        def zero_qkplane(b4):
            # zero plane-1 of K^T and Q^T: the DoubleRow second plane must
            # contract to 0, and junk fp8 bytes can decode to NaN (0*NaN=NaN).
            # Split per token block so the front isn't serialized on one big
            # DVE memset.
            nc.vector.memset(KT8[:, :, 1, ts(b4, 512)], 0.0)
            nc.vector.memset(QT8[:, :, 1, ts(b4, 512)], 0.0)

        # ---------------- weight + x staging
        # x rides the single serial SWDGE cast queue; weights go via HWDGE
        # (fp32) + DVE casts, and the fp8 copies are made on the idle Pool
        # engine, so the big casts don't serialize behind each other.
        xv = x.rearrange("(b t p) dm -> b p t dm", p=128, t=4)

        def stage_xblock(b4):
            xbf4 = sp.tile([128, 4, DMc], BF16, tag="xbf", name="xbf4")
            nc.gpsimd.dma_start(xbf4, xv[b4])
            for t4 in range(4):
                nc.sync.dma_start(
                    xT[:, :, ts(4 * b4 + t4, 128)], xbf4[:, t4, :], transpose=True
                )
            if cfg.FP8_PROJ and b4 > 0:
                nc.gpsimd.tensor_copy(x8T[:, :, ts(b4, 512)],
                                      xT[:, :, ts(b4, 512)])
            if b4 > 0:
                zero_qkplane(b4)

        emit_consts()
        zero_qkplane(0)
        # wqkv: per-chunk HWDGE fp32 loads; bf16 copy casts on DVE and the
        # fp8 q|k copy casts on Pool, each as soon as its chunk lands.
        # Emitted before the x transposes so the weight chunks aren't stuck
        # behind transpose waits in the in-order HWDGE issue stream.
        wqv = wqkv.rearrange("(c p) f -> p c f", p=128)
        for c in range(NDM):
            wst = sp4.tile([128, 1, 3 * FLOCc], F32, tag="wst", name="wst")
            nc.sync.dma_start(wst, wqv[:, c : c + 1, :])
            nc.vector.tensor_copy(wqkvb[:, c : c + 1, :], wst)
            if cfg.FP8_PROJ:
                nc.gpsimd.tensor_copy(wqk8[:, c : c + 1, :],
                                      wst[:, :, 0 : 2 * FLOCc])
        stage_xblock(0)
        if cfg.FP8_PROJ:
            nc.vector.tensor_copy(x8T[:, :, ts(0, 512)], xT[:, :, ts(0, 512)])
        for b4 in range(1, NB):
            stage_xblock(b4)
        wov = wo.rearrange("(c p) f -> p c f", p=128)
        for cc in range(NDM // 2):
            wst2 = sp.tile([128, 2, FLOCc], F32, tag="wst2", name="wst2")
            nc.sync.dma_start(wst2, wov[:, 2 * cc : 2 * cc + 2, :])
            nc.vector.tensor_copy(wob[:, 2 * cc : 2 * cc + 2, :], wst2)

        # ---------------- QKV projection units
        def emit_qk_unit(b, which, ft):
            dst, bias = ((QT8, bq_f) if which == 0 else (KT8, bk_f))
            qk = psum.tile([128, 512], F32, tag="work", name="qk")
            if cfg.FP8_PROJ:
                w8 = wqk8[:, :, which * FLOCc :][:, :, ts(ft, 128)]
                for cc in range(NDM // 2):
                    nc.tensor.matmul(
                        qk, w8[:, 2 * cc : 2 * cc + 2, :],
                        x8T[:, 2 * cc : 2 * cc + 2, ts(b, 512)],
                        start=(cc == 0), stop=(cc == NDM // 2 - 1), perf_mode=DR,
                    )
            else:
                wsl = wqb if which == 0 else wkb
                for c in range(NDM):
                    nc.tensor.matmul(
                        qk, wsl[:, c, ts(ft, 128)], xT[:, c, ts(b, 512)],
                        start=(c == 0), stop=(c == NDM - 1),
                    )
            nc.vector.tensor_scalar_add(dst[:, ft, 0, ts(b, 512)], qk,
                                        bias[:, ft : ft + 1])

        def emit_v_unit(b, sub):
            tt = 4 * b + sub
            psv_full = psum.tile([128, 512], F32, tag="work", name="psv")
            psv = psv_full[:, 0:FLOCc]
            for c in range(NDM):
                nc.tensor.matmul(
                    psv, xT[:, c, ts(tt, 128)], wvb[:, c, :],
                    start=(c == 0), stop=False,
                )
            nc.tensor.matmul(psv, ones_r, bvb, start=False, stop=True)
            nc.vector.tensor_copy(
                Vb[:, tt, :].rearrange("p (h u) -> p h u", u=HDc + 1)[:, :, 0:HDc],
                psv.rearrange("p (h d) -> p h d", d=HDc),
            )

        def qkv_units(b, v_first=False):
            qk = [(lambda w=w, i=i, b=b: emit_qk_unit(b, w, i))
                  for w in (0, 1) for i in (0, 1)]
            v = [(lambda s=s, b=b: emit_v_unit(b, s)) for s in range(4)]
            return v + qk if v_first else qk + v

        for u in qkv_units(0, v_first=True):
            u()

        # ---------------- attention + allgather + out projection
        def emit_scores(p, h, pi):
            """Scores for k-tiles (2*pi, 2*pi+1) in one [128,1024] PSUM tile."""
            hf, hp = h // 2, h % 2
            S = psum.tile([128, 1024], F32, tag="stile", name="S")
            for s in range(2):
                i = 2 * pi + s
                al = max(0, 128 * i - 512 * p)
                if cfg.USE_FP8:
                    nc.tensor.matmul(
                        S[:, ds(512 * s + al, 512 - al)],
                        KT8[64 * hp : 64 * hp + 64, hf, :, ts(i, 128)],
                        QT8[64 * hp : 64 * hp + 64, hf, :, ds(512 * p + al, 512 - al)],
                        start=True, stop=True, perf_mode=DR,
                    )
                else:
                    nc.tensor.matmul(
                        S[:, ds(512 * s + al, 512 - al)],
                        KT8[64 * hp : 64 * hp + 64, hf, 0, ts(i, 128)],
                        QT8[64 * hp : 64 * hp + 64, hf, 0, ds(512 * p + al, 512 - al)],
                        start=True, stop=True,
                    )
            return S

        ag_outs = {}

        def emit_ag(key, q0, qw):
            ag_in = dramp.tile([NFT * 128, qw], BF16, tag=f"agin{qw}", name="ag_in")
            ag_out = dramp.tile([GRP * NFT * 128, qw], BF16, tag=f"agout{qw}",
                                name="ag_out")
            for t in range(NFT):
                nc.sync.dma_start(ag_in[ts(t, 128), :], OTs[:, t, ds(q0, qw)])
            nc.gpsimd.collective_compute(
                "AllGather",
                mybir.AluOpType.bypass,
                ins=[ag_in.opt()],
                outs=[ag_out.opt()],
                replica_groups=groups,
            )
            ag_outs[key] = (ag_out, q0, qw)

        of_tiles = {}

        def emit_of_load(key):
            ag_out, q0, qw = ag_outs[key]
            OF = ofp.tile([128, NDM, 512], BF16, tag="of", name="OF")
            agv = ag_out.rearrange("(c p) q -> c p q", p=128)
            for c in range(NDM):
                nc.sync.dma_start(OF[:, c, 0:qw], agv[c])
            of_tiles[key] = (OF, osbp.tile([128, 4, FLOCc], F32,
                                           tag="osb", name="osb"), q0, qw)

        def emit_op_unit(key, ttl):
            OF, osb, q0, qw = of_tiles[key]
            ntl = qw // 128
            pout_full = psum.tile([128, 512], F32, tag="work", name="pout")
            pout = pout_full[:, 0:FLOCc]
            for c in range(NDM):
                nc.tensor.matmul(
                    pout, OF[:, c, ts(ttl, 128)], wob[:, c, :],
                    start=(c == 0), stop=False,
                )
            nc.tensor.matmul(pout, ones_r, bob, start=False, stop=True)
            nc.vector.tensor_copy(osb[:, ttl, :], pout)
            outv = out[ds(q0, qw), :].rearrange("(t p) f -> p t f", p=128)
            half = ntl // 2
            if ttl == half - 1:
                nc.sync.dma_start(outv[:, 0:half, :], osb[:, 0:half, :])
            if ttl == ntl - 1 and ntl > half:
                nc.sync.dma_start(outv[:, half:ntl, :], osb[:, half:ntl, :])

        def emit_pv_chain(p, h, Ps, j, Otm):
            """One qtile's full accumulation chain (consecutive matmuls into a
            dedicated PSUM bank), then normalize; transpose on the last head."""
            jg = 4 * p + j
            accb = psum.tile([128, 512], F32, tag="acc", name="acc")
            acc = accb[:, 0 : HDc + 1]
            for i in range(jg + 1):
                pi, s = divmod(i, 2)
                nc.tensor.matmul(
                    acc, Ps[pi][:, ds(512 * s + 128 * j, 128)],
                    Vb[:, i, ds((HDc + 1) * h, HDc + 1)],
                    start=(i == 0), stop=(i == jg),
                )
            rec = nrm.tile([128, 1], F32, tag="rec", name="rec")
            nc.vector.reciprocal(rec, accb[:, HDc : HDc + 1])
            nc.vector.tensor_scalar_mul(
                Otm[:, j, ds(HDc * h, HDc)], accb[:, 0:HDc], rec)
            if h == HLOCc - 1:
                nc.sync.dma_start(OTs[:, :, ts(jg, 128)],
                                  Otm[:, j, :], transpose=True)

        for p in range(NPASS):
            npair = 2 * p + 2
            nslot = HLOCc * npair

            units = []
            if p < NPASS - 1:
                units += qkv_units(p + 1)
            if p == 1:
                units.append(lambda: emit_of_load(0))
                units += [(lambda t=t: emit_op_unit(0, t)) for t in range(4)]
            if p == 3:
                units.append(lambda: emit_of_load(1))
                units += [(lambda t=t: emit_op_unit(1, t)) for t in range(4)]
                units.append(lambda: emit_of_load(2))
                units += [(lambda t=t: emit_op_unit(2, t)) for t in range(4)]
            upos = {}
            for k, u in enumerate(units):
                upos.setdefault(1 + (k * (nslot - 2)) // max(1, len(units) - 1),
                                []).append(u)

            Otm = otmp.tile([128, 4, FLOCc], BF16, tag="otm", name="Otm")
            prevPs = None
            slot = 0
            for h in range(HLOCc):
                Ps = []
                for pi in range(npair):
                    S = emit_scores(p, h, pi)
                    al0 = max(0, 128 * (2 * pi) - 512 * p)
                    P = pbp.tile([128, 1024], BF16, tag="ptile", name="P")
                    nc.scalar.activation(
                        P[:, ds(al0, 1024 - al0)], S[:, ds(al0, 1024 - al0)],
                        mybir.ActivationFunctionType.Exp, scale=float(cfg.scale),
                    )
                    for s in range(2):
                        i = 2 * pi + s
                        if i >= 4 * p:  # diagonal block
                            off = 512 * s + 128 * (i - 4 * p)
                            nc.vector.tensor_mul(P[:, ds(off, 128)],
                                                 P[:, ds(off, 128)], trimask)
                    Ps.append(P)
                    for fn in upos.get(slot, ()):
                        fn()
                    if prevPs is not None and pi < 4:
                        emit_pv_chain(p, h - 1, prevPs, pi, Otm)
                    slot += 1
                if prevPs is not None:
                    for j in range(npair, 4):
                        emit_pv_chain(p, h - 1, prevPs, j, Otm)
                prevPs = Ps
            if p < NPASS - 1:
                for j in range(4):
                    emit_pv_chain(p, HLOCc - 1, prevPs, j, Otm)
                emit_ag(p, 512 * p, 512)
            else:
                # last pass: gather + out-project in two halves so the tail
                # after the final normalize is as short as possible
                for j in range(2):
                    emit_pv_chain(p, HLOCc - 1, prevPs, j, Otm)
                emit_ag("3a", 512 * p, 256)
                for j in range(2, 4):
                    emit_pv_chain(p, HLOCc - 1, prevPs, j, Otm)
                emit_ag("3b", 512 * p + 256, 256)
                emit_of_load("3a")
                for t in range(2):
                    emit_op_unit("3a", t)
                emit_of_load("3b")
                for t in range(2):
                    emit_op_unit("3b", t)
        if dbg_hook is not None:
            dbg_hook(locals())
        psum_cm.__exit__(None, None, None)


def make_program(cfg=None, groups=None, unroll=1):
    cfg = cfg or Cfg()
    groups = groups or REPLICA_GROUPS
    nc = bacc.Bacc("TRN2", target_bir_lowering=False, debug=False, num_devices=NCORES)
    x = nc.dram_tensor("x", [cfg.L, cfg.DM], F32, kind="ExternalInput").ap()
    wqkv = nc.dram_tensor("wqkv", [cfg.DM, 3 * cfg.FLOC], F32, kind="ExternalInput").ap()
    bq = nc.dram_tensor("bq", [cfg.FLOC], F32, kind="ExternalInput").ap()
    bk = nc.dram_tensor("bk", [cfg.FLOC], F32, kind="ExternalInput").ap()
    bv = nc.dram_tensor("bv", [cfg.FLOC], F32, kind="ExternalInput").ap()
    wo = nc.dram_tensor("wo", [cfg.DM, cfg.FLOC], F32, kind="ExternalInput").ap()
    bo = nc.dram_tensor("bo", [cfg.FLOC], F32, kind="ExternalInput").ap()
    out = nc.dram_tensor("out", [cfg.L, cfg.FLOC], F32, kind="ExternalOutput").ap()
    with tile.TileContext(nc) as tc:
        nc.tc = tc
        for _ in range(unroll):
            build_body(nc, cfg, x, wqkv, bq, bk, bv, wo, bo, out, groups)
    nc.compile()
    return nc


def shard_inputs(x, w_qkv, b_qkv, w_out, b_out, cfg=None):
    """Full inputs -> list of 8 per-core input dicts."""
    cfg = cfg or Cfg()
    FL = cfg.FLOC
    DMF = cfg.DM
    in_maps = []
    for c in range(NCORES):
        b, r = divmod(c, GRP)
        q0 = r * FL
        in_maps.append({
            "x": np.ascontiguousarray(x[b]),
            "wqkv": np.ascontiguousarray(np.concatenate([
                w_qkv[:, q0 : q0 + FL],
                w_qkv[:, DMF + q0 : DMF + q0 + FL],
                w_qkv[:, 2 * DMF + q0 : 2 * DMF + q0 + FL],
            ], axis=1)),
            "bq": np.ascontiguousarray(b_qkv[q0 : q0 + FL]),
            "bk": np.ascontiguousarray(b_qkv[DMF + q0 : DMF + q0 + FL]),
            "bv": np.ascontiguousarray(b_qkv[2 * DMF + q0 : 2 * DMF + q0 + FL]),
            "wo": np.ascontiguousarray(w_out[:, q0 : q0 + FL]),
            "bo": np.ascontiguousarray(b_out[q0 : q0 + FL]),
        })
    return in_maps


def gather_output(results, cfg=None):
    cfg = cfg or Cfg()
    FL = cfg.FLOC
    out = np.empty((BS, cfg.L, cfg.DM), np.float32)
    for c in range(NCORES):
        b, r = divmod(c, GRP)
        out[b, :, r * FL : (r + 1) * FL] = results[c]["out"]
    return out


_PROGRAM = None


def _get_program():
    global _PROGRAM
    if _PROGRAM is None:
        _PROGRAM = make_program()
    return _PROGRAM


def kernel(x, w_qkv, b_qkv, w_out, b_out):
    x = np.asarray(x, np.float32)
    w_qkv = np.asarray(w_qkv, np.float32)
    b_qkv = np.asarray(b_qkv, np.float32)
    w_out = np.asarray(w_out, np.float32)
    b_out = np.asarray(b_out, np.float32)
    nc = _get_program()
    in_maps = shard_inputs(x, w_qkv, b_qkv, w_out, b_out)
    res = run_bass_kernel_spmd(nc, in_maps, list(range(NCORES)))
    return gather_output(res.results)


# revision 33
# speedup vs baseline: 1.0104x; 1.0104x over previous
"""Trainium2 Bass kernel for causal multi-head attention (dense transformer block).

Problem: x[2,2048,1024] -> qkv proj -> 16-head causal attention (scale 1/sqrt(1024))
         -> out proj.  8 NeuronCores.

Sharding: core c handles batch b=c//4 and head-group r=c%4 (heads 4r..4r+3).
  - qkv weights column-sharded by head group (q/k/v slices of 256 cols each)
  - Q/K projection runs in fp8e4 DoubleRow (dm-chunk pairs as the second
    contraction plane, 0.5 cycles/row); Q/K are only consumed by the fp8
    score matmul, so the extra quantization is cheap in accuracy.
  - S^T[k,q] = K^T (stationary) x Q^T (moving), fp8e4 DoubleRow with a
    zeroed second plane (head_dim 64 lives on 64 partitions; plane 1 of K^T
    is zeros so it contracts to 0).  Scores for two k-tiles share one PSUM
    tile so one exp() activation covers both (halves the Act-engine's fixed
    per-instruction SBUF-access overhead).
  - P = exp(S/32) (bf16) with causal masking; attention output accumulated
    token-major: acc[q, 65] += P_blk^T x [V|1] per k-tile (PSUM partition dim
    stays 128; the softmax denominator is a free column, normalized by a
    per-partition reciprocal multiply on the DVE).
  - The out-proj follows an AllGather of O^T (bf16, groups of 4 cores).
  - 4-pass software pipeline: QKV projection for token-block p+1 and deferred
    out-proj work are spliced into pass p's attention job stream so the PE
    stays busy while the Activation engine works through exp().

kernel(**inputs) takes the FULL fp32 inputs and returns the FULL output.
"""

import sys

sys.path.insert(0, "/opt/trn_rl_repo")

import numpy as np

import concourse.bass as bass
import concourse.bacc as bacc
import concourse.mybir as mybir
import concourse.tile as tile
from concourse.bass import ds, ts
from concourse.bass_utils import run_bass_kernel_spmd
from concourse.masks import make_upper_triangular

F32 = mybir.dt.float32
BF16 = mybir.dt.bfloat16
FP8 = mybir.dt.float8e4

# ---------------------------------------------------------------- dims
BS, L, DM, H = 2, 2048, 1024, 16
HD = 64                      # head dim
NCORES = 8
GRP = 4                      # cores per batch group (head-parallel)
HLOC = H // GRP              # heads per core = 4
FLOC = HLOC * HD             # local features = 256
SCALE = 1.0 / float(np.sqrt(DM))
REPLICA_GROUPS = [[0, 1, 2, 3], [4, 5, 6, 7]]


class Cfg:
    def __init__(self, L=L, DM=DM, hloc=HLOC, hd=HD, npass=4, nwarm=32,
                 use_fp8=True, fp8_proj=True):
        self.L, self.DM, self.HLOC, self.HD, self.NPASS = L, DM, hloc, hd, npass
        self.FLOC = hloc * hd
        self.NT = L // 128           # 128-token tiles
        self.NB = L // 512           # 512-token blocks
        self.NDM = DM // 128         # dmodel chunks
        self.PW = L // npass         # pass width (q columns per pass)
        self.NFT = self.FLOC // 128  # feature tiles for O^T (2)
        self.NWARM = nwarm
        self.USE_FP8 = use_fp8
        self.FP8_PROJ = fp8_proj and use_fp8
        self.scale = 1.0 / float(np.sqrt(DM))
        assert self.PW == 512 and self.FLOC % 128 == 0


def build_body(nc, cfg, x, wqkv, bq, bk, bv, wo, bo, out, groups, dbg_hook=None):
    """Emit the per-core program (Tile framework)."""
    NT, NB, NDM, NFT = cfg.NT, cfg.NB, cfg.NDM, cfg.NFT
    HLOCc, HDc, FLOCc = cfg.HLOC, cfg.HD, cfg.FLOC
    Lc, DMc = cfg.L, cfg.DM
    NPASS = cfg.NPASS
    QKDT = FP8 if cfg.USE_FP8 else BF16
    tc = nc.tc
    DR = mybir.MatmulPerfMode.DoubleRow

    with tc.tile_pool(name="const", bufs=1) as constp, \
         tc.tile_pool(name="persist", bufs=1) as pp, \
         tc.tile_pool(name="stage", bufs=2) as sp, \
         tc.tile_pool(name="stage4", bufs=4) as sp4, \
         tc.tile_pool(name="pbuf", bufs=13) as pbp, \
         tc.tile_pool(name="nrm", bufs=4) as nrm, \
         tc.tile_pool(name="otm", bufs=2) as otmp, \
         tc.tile_pool(name="of", bufs=2) as ofp, \
         tc.tile_pool(name="osb", bufs=2) as osbp, \
         tc.tile_pool(name="dram", bufs=2, space="DRAM") as dramp:
        # ---------------- persistent SBUF tensors
        xT = pp.tile([128, NDM, Lc], BF16)                 # x^T  (dm-major)
        wqkvb = pp.tile([128, NDM, 3 * FLOCc], BF16)       # [wq|wk|wv] packed
        wqb = wqkvb[:, :, 0:FLOCc]
        wkb = wqkvb[:, :, FLOCc : 2 * FLOCc]
        wvb = wqkvb[:, :, 2 * FLOCc : 3 * FLOCc]
        wob = pp.tile([128, NDM, FLOCc], BF16)
        if cfg.FP8_PROJ:
            x8T = pp.tile([128, NDM, Lc], FP8)             # fp8 copy of x^T
            wqk8 = pp.tile([128, NDM, 2 * FLOCc], FP8)     # fp8 [wq|wk]
        # Q^T/K^T feature-major with a DoubleRow plane dim: chunk hf holds
        # heads 2hf,2hf+1 (partition 64*(h%2)+hd); plane 0 = data, plane 1 =
        # zeros so the fp8 DoubleRow matmul contracts (K,Q) + (0,0).
        QT8 = pp.tile([128, 2, 2, Lc], QKDT)
        KT8 = pp.tile([128, 2, 2, Lc], QKDT)
        Vb = pp.tile([128, NT, HLOCc * (HDc + 1)], BF16)   # [V | ones] per token tile
        OTs = pp.tile([128, NFT, Lc], BF16)                # attention out^T (feature-major)

        # ---------------- single PSUM pool for the whole kernel
        # banks: stile [128,1024] x2 = 4, acc [128,512] x2 = 2,
        #        work [128,512] x2 = 2  -> 8 banks
        # NOTE: matmul start=True marks the enclosing 2KB zero-region pending,
        # so each accumulation chain owns a full bank (acc padded to 512 f32)
        # and runs its matmuls consecutively.
        psum_cm = tc.tile_pool(name="psum", bufs=2, space="PSUM")
        psum = psum_cm.__enter__()

        # PE warmup: junk matmuls so the p-state ramp happens on the DMA-bound
        # front, not on the first real matmuls.
        wsrc_t = pp.tile([128, 512], BF16, name="wsrc_t")
        nc.vector.memset(wsrc_t, 0.25)
        wps = psum.tile([128, 512], F32, tag="work", name="wps")
        for r in range(cfg.NWARM):
            nc.tensor.matmul(wps, wsrc_t[:, 0:128], wsrc_t,
                             start=(r == 0), stop=(r == cfg.NWARM - 1))
        wout_t = pp.tile([128, 512], F32, name="wout_t")
        nc.vector.tensor_copy(wout_t, wps)
        # preload the Exp activation table during the front
        wexp_t = pp.tile([1, 1], BF16, name="wexp_t")
        nc.scalar.activation(wexp_t, wsrc_t[0:1, 0:1],
                             mybir.ActivationFunctionType.Exp)

        # ---------------- constants
        trimask = constp.tile([128, 128], BF16)
        ones_r = constp.tile([1, 128], BF16)
        bq_f = constp.tile([128, 2], F32)
        bk_f = constp.tile([128, 2], F32)
        bvb = constp.tile([1, FLOCc], BF16)
        bob = constp.tile([1, FLOCc], BF16)

        def emit_consts():
            make_upper_triangular(nc, trimask, val=1.0, diag=True)
            nc.vector.memset(ones_r, 1.0)
            nc.sync.dma_start(bq_f, bq.rearrange("(f p) -> p f", p=128))
            nc.sync.dma_start(bk_f, bk.rearrange("(f p) -> p f", p=128))
            bv_st = constp.tile([1, 2 * FLOCc], F32, name="bv_st")
            nc.sync.dma_start(bv_st[:, 0:FLOCc], bv.rearrange("(a b) -> a b", a=1))
            nc.sync.dma_start(bv_st[:, FLOCc : 2 * FLOCc], bo.rearrange("(a b) -> a b", a=1))
            nc.vector.tensor_copy(bvb, bv_st[:, 0:FLOCc])
            nc.vector.tensor_copy(bob, bv_st[:, FLOCc : 2 * FLOCc])
            nc.vector.memset(
                Vb.rearrange("p t (h u) -> p t h u", u=HDc + 1)[:, :, :, HDc : HDc + 1], 1.0
            )

        def zero_qkplane(b4):
            # zero plane-1 of K^T and Q^T: the DoubleRow second plane must
            # contract to 0, and junk fp8 bytes can decode to NaN (0*NaN=NaN).
            # Split per token block so the front isn't serialized on one big
            # DVE memset.
            nc.vector.memset(KT8[:, :, 1, ts(b4, 512)], 0.0)
            nc.vector.memset(QT8[:, :, 1, ts(b4, 512)], 0.0)

        # ---------------- weight + x staging
        # x rides the single serial SWDGE cast queue; weights go via HWDGE
        # (fp32) + DVE casts, and the fp8 copies are made on the idle Pool
        # engine, so the big casts don't serialize behind each other.
        xv = x.rearrange("(b t p) dm -> b p t dm", p=128, t=4)

        xbfs = {}

        def stage_x_dma(b4):
            xbfs[b4] = sp.tile([128, 4, DMc], BF16, tag="xbf", name="xbf4")
            nc.gpsimd.dma_start(xbfs[b4], xv[b4])

        def stage_x_transpose(b4):
            for t4 in range(4):
                nc.sync.dma_start(
                    xT[:, :, ts(4 * b4 + t4, 128)], xbfs[b4][:, t4, :],
                    transpose=True,
                )
            if cfg.FP8_PROJ:
                # block 0 casts on DVE (front); later blocks on Pool
                if b4 == 0:
                    nc.vector.tensor_copy(x8T[:, :, ts(b4, 512)],
                                          xT[:, :, ts(b4, 512)])
                else:
                    nc.gpsimd.tensor_copy(x8T[:, :, ts(b4, 512)],
                                          xT[:, :, ts(b4, 512)])
            if b4 > 0:
                zero_qkplane(b4)

        # x block 0's cast-DMA first (heads the serial SWDGE chain), then the
        # tiny const DMAs, then the weight chunks (HWDGE, no deps), then the
        # block-0 transposes (which wait on x0), then the remaining x blocks.
        stage_x_dma(0)
        emit_consts()
        zero_qkplane(0)
        wqv = wqkv.rearrange("(c p) f -> p c f", p=128)
        for c in range(NDM):
            wst = sp4.tile([128, 1, 3 * FLOCc], F32, tag="wst", name="wst")
            nc.sync.dma_start(wst, wqv[:, c : c + 1, :])
            nc.vector.tensor_copy(wqkvb[:, c : c + 1, :], wst)
            if cfg.FP8_PROJ:
                nc.vector.tensor_copy(wqk8[:, c : c + 1, :],
                                      wst[:, :, 0 : 2 * FLOCc])
        stage_x_transpose(0)
        for b4 in range(1, NB):
            stage_x_dma(b4)
            stage_x_transpose(b4)
        wov = wo.rearrange("(c p) f -> p c f", p=128)
        for cc in range(NDM // 2):
            wst2 = sp.tile([128, 2, FLOCc], F32, tag="wst2", name="wst2")
            nc.sync.dma_start(wst2, wov[:, 2 * cc : 2 * cc + 2, :])
            nc.vector.tensor_copy(wob[:, 2 * cc : 2 * cc + 2, :], wst2)

        # ---------------- QKV projection units
        def emit_qk_unit(b, which, ft):
            dst, bias = ((QT8, bq_f) if which == 0 else (KT8, bk_f))
            qk = psum.tile([128, 512], F32, tag="work", name="qk")
            if cfg.FP8_PROJ:
                w8 = wqk8[:, :, which * FLOCc :][:, :, ts(ft, 128)]
                for cc in range(NDM // 2):
                    nc.tensor.matmul(
                        qk, w8[:, 2 * cc : 2 * cc + 2, :],
                        x8T[:, 2 * cc : 2 * cc + 2, ts(b, 512)],
                        start=(cc == 0), stop=(cc == NDM // 2 - 1), perf_mode=DR,
                    )
            else:
                wsl = wqb if which == 0 else wkb
                for c in range(NDM):
                    nc.tensor.matmul(
                        qk, wsl[:, c, ts(ft, 128)], xT[:, c, ts(b, 512)],
                        start=(c == 0), stop=(c == NDM - 1),
                    )
            nc.vector.tensor_scalar_add(dst[:, ft, 0, ts(b, 512)], qk,
                                        bias[:, ft : ft + 1])

        def emit_v_unit(b, sub):
            tt = 4 * b + sub
            psv_full = psum.tile([128, 512], F32, tag="work", name="psv")
            psv = psv_full[:, 0:FLOCc]
            for c in range(NDM):
                nc.tensor.matmul(
                    psv, xT[:, c, ts(tt, 128)], wvb[:, c, :],
                    start=(c == 0), stop=False,
                )
            nc.tensor.matmul(psv, ones_r, bvb, start=False, stop=True)
            nc.vector.tensor_copy(
                Vb[:, tt, :].rearrange("p (h u) -> p h u", u=HDc + 1)[:, :, 0:HDc],
                psv.rearrange("p (h d) -> p h d", d=HDc),
            )

        def qkv_units(b, v_first=False):
            qk = [(lambda w=w, i=i, b=b: emit_qk_unit(b, w, i))
                  for w in (0, 1) for i in (0, 1)]
            v = [(lambda s=s, b=b: emit_v_unit(b, s)) for s in range(4)]
            return v + qk if v_first else qk + v

        for u in qkv_units(0, v_first=True):
            u()

        # ---------------- attention + allgather + out projection
        def emit_scores(p, h, pi):
            """Scores for k-tiles (2*pi, 2*pi+1) in one [128,1024] PSUM tile."""
            hf, hp = h // 2, h % 2
            S = psum.tile([128, 1024], F32, tag="stile", name="S")
            for s in range(2):
                i = 2 * pi + s
                al = max(0, 128 * i - 512 * p)
                if cfg.USE_FP8:
                    nc.tensor.matmul(
                        S[:, ds(512 * s + al, 512 - al)],
                        KT8[64 * hp : 64 * hp + 64, hf, :, ts(i, 128)],
                        QT8[64 * hp : 64 * hp + 64, hf, :, ds(512 * p + al, 512 - al)],
                        start=True, stop=True, perf_mode=DR,
                    )
                else:
                    nc.tensor.matmul(
                        S[:, ds(512 * s + al, 512 - al)],
                        KT8[64 * hp : 64 * hp + 64, hf, 0, ts(i, 128)],
                        QT8[64 * hp : 64 * hp + 64, hf, 0, ds(512 * p + al, 512 - al)],
                        start=True, stop=True,
                    )
            return S

        ag_outs = {}

        def emit_ag(key, q0, qw):
            ag_in = dramp.tile([NFT * 128, qw], BF16, tag=f"agin{qw}", name="ag_in")
            ag_out = dramp.tile([GRP * NFT * 128, qw], BF16, tag=f"agout{qw}",
                                name="ag_out")
            for t in range(NFT):
                nc.sync.dma_start(ag_in[ts(t, 128), :], OTs[:, t, ds(q0, qw)])
            nc.gpsimd.collective_compute(
                "AllGather",
                mybir.AluOpType.bypass,
                ins=[ag_in.opt()],
                outs=[ag_out.opt()],
                replica_groups=groups,
            )
            ag_outs[key] = (ag_out, q0, qw)

        of_tiles = {}

        def emit_of_load(key):
            ag_out, q0, qw = ag_outs[key]
            OF = ofp.tile([128, NDM, 512], BF16, tag="of", name="OF")
            agv = ag_out.rearrange("(c p) q -> c p q", p=128)
            for c in range(NDM):
                nc.sync.dma_start(OF[:, c, 0:qw], agv[c])
            of_tiles[key] = (OF, osbp.tile([128, 4, FLOCc], F32,
                                           tag="osb", name="osb"), q0, qw)

        def emit_op_unit(key, ttl):
            OF, osb, q0, qw = of_tiles[key]
            ntl = qw // 128
            pout_full = psum.tile([128, 512], F32, tag="work", name="pout")
            pout = pout_full[:, 0:FLOCc]
            for c in range(NDM):
                nc.tensor.matmul(
                    pout, OF[:, c, ts(ttl, 128)], wob[:, c, :],
                    start=(c == 0), stop=False,
                )
            nc.tensor.matmul(pout, ones_r, bob, start=False, stop=True)
            nc.vector.tensor_copy(osb[:, ttl, :], pout)
            outv = out[ds(q0, qw), :].rearrange("(t p) f -> p t f", p=128)
            half = ntl // 2
            if ttl == half - 1:
                nc.sync.dma_start(outv[:, 0:half, :], osb[:, 0:half, :])
            if ttl == ntl - 1 and ntl > half:
                nc.sync.dma_start(outv[:, half:ntl, :], osb[:, half:ntl, :])

        def emit_pv_chain(p, h, Ps, j, Otm):
            """One qtile's full accumulation chain (consecutive matmuls into a
            dedicated PSUM bank), then normalize; transpose on the last head."""
            jg = 4 * p + j
            accb = psum.tile([128, 512], F32, tag="acc", name="acc")
            acc = accb[:, 0 : HDc + 1]
            for i in range(jg + 1):
                pi, s = divmod(i, 2)
                nc.tensor.matmul(
                    acc, Ps[pi][:, ds(512 * s + 128 * j, 128)],
                    Vb[:, i, ds((HDc + 1) * h, HDc + 1)],
                    start=(i == 0), stop=(i == jg),
                )
            rec = nrm.tile([128, 1], F32, tag="rec", name="rec")
            nc.vector.reciprocal(rec, accb[:, HDc : HDc + 1])
            nc.vector.tensor_scalar_mul(
                Otm[:, j, ds(HDc * h, HDc)], accb[:, 0:HDc], rec)
            if h == HLOCc - 1:
                nc.sync.dma_start(OTs[:, :, ts(jg, 128)],
                                  Otm[:, j, :], transpose=True)

        for p in range(NPASS):
            npair = 2 * p + 2
            nslot = HLOCc * npair

            units = []
            if p < NPASS - 1:
                units += qkv_units(p + 1)
            if p == 1:
                units.append(lambda: emit_of_load(0))
                units += [(lambda t=t: emit_op_unit(0, t)) for t in range(4)]
            if p == 3:
                units.append(lambda: emit_of_load(1))
                units += [(lambda t=t: emit_op_unit(1, t)) for t in range(4)]
                units.append(lambda: emit_of_load(2))
                units += [(lambda t=t: emit_op_unit(2, t)) for t in range(4)]
            upos = {}
            for k, u in enumerate(units):
                upos.setdefault(1 + (k * (nslot - 2)) // max(1, len(units) - 1),
                                []).append(u)

            Otm = otmp.tile([128, 4, FLOCc], BF16, tag="otm", name="Otm")
            prevPs = None
            slot = 0
            for h in range(HLOCc):
                Ps = []
                for pi in range(npair):
                    S = emit_scores(p, h, pi)
                    al0 = max(0, 128 * (2 * pi) - 512 * p)
                    P = pbp.tile([128, 1024], BF16, tag="ptile", name="P")
                    nc.scalar.activation(
                        P[:, ds(al0, 1024 - al0)], S[:, ds(al0, 1024 - al0)],
                        mybir.ActivationFunctionType.Exp, scale=float(cfg.scale),
                    )
                    for s in range(2):
                        i = 2 * pi + s
                        if i >= 4 * p:  # diagonal block
                            off = 512 * s + 128 * (i - 4 * p)
                            nc.vector.tensor_mul(P[:, ds(off, 128)],
                                                 P[:, ds(off, 128)], trimask)
                    Ps.append(P)
                    for fn in upos.get(slot, ()):
                        fn()
                    if prevPs is not None and pi < 4:
                        emit_pv_chain(p, h - 1, prevPs, pi, Otm)
                    slot += 1
                if prevPs is not None:
                    for j in range(npair, 4):
                        emit_pv_chain(p, h - 1, prevPs, j, Otm)
                prevPs = Ps
            if p < NPASS - 1:
                for j in range(4):
                    emit_pv_chain(p, HLOCc - 1, prevPs, j, Otm)
                emit_ag(p, 512 * p, 512)
            else:
                # last pass: gather + out-project in two halves so the tail
                # after the final normalize is as short as possible
                for j in range(2):
                    emit_pv_chain(p, HLOCc - 1, prevPs, j, Otm)
                emit_ag("3a", 512 * p, 256)
                for j in range(2, 4):
                    emit_pv_chain(p, HLOCc - 1, prevPs, j, Otm)
                emit_ag("3b", 512 * p + 256, 256)
                emit_of_load("3a")
                for t in range(2):
                    emit_op_unit("3a", t)
                emit_of_load("3b")
                for t in range(2):
                    emit_op_unit("3b", t)
        if dbg_hook is not None:
            dbg_hook(locals())
        psum_cm.__exit__(None, None, None)


def make_program(cfg=None, groups=None, unroll=1):
    cfg = cfg or Cfg()
    groups = groups or REPLICA_GROUPS
    nc = bacc.Bacc("TRN2", target_bir_lowering=False, debug=False, num_devices=NCORES)
    x = nc.dram_tensor("x", [cfg.L, cfg.DM], F32, kind="ExternalInput").ap()
    wqkv = nc.dram_tensor("wqkv", [cfg.DM, 3 * cfg.FLOC], F32, kind="ExternalInput").ap()
    bq = nc.dram_tensor("bq", [cfg.FLOC], F32, kind="ExternalInput").ap()
    bk = nc.dram_tensor("bk", [cfg.FLOC], F32, kind="ExternalInput").ap()
    bv = nc.dram_tensor("bv", [cfg.FLOC], F32, kind="ExternalInput").ap()
    wo = nc.dram_tensor("wo", [cfg.DM, cfg.FLOC], F32, kind="ExternalInput").ap()
    bo = nc.dram_tensor("bo", [cfg.FLOC], F32, kind="ExternalInput").ap()
    out = nc.dram_tensor("out", [cfg.L, cfg.FLOC], F32, kind="ExternalOutput").ap()
    with tile.TileContext(nc) as tc:
        nc.tc = tc
        for _ in range(unroll):
            build_body(nc, cfg, x, wqkv, bq, bk, bv, wo, bo, out, groups)
    nc.compile()
    return nc


def shard_inputs(x, w_qkv, b_qkv, w_out, b_out, cfg=None):
    """Full inputs -> list of 8 per-core input dicts."""
    cfg = cfg or Cfg()
    FL = cfg.FLOC
    DMF = cfg.DM
    in_maps = []
    for c in range(NCORES):
        b, r = divmod(c, GRP)
        q0 = r * FL
        in_maps.append({
            "x": np.ascontiguousarray(x[b]),
            "wqkv": np.ascontiguousarray(np.concatenate([
                w_qkv[:, q0 : q0 + FL],
                w_qkv[:, DMF + q0 : DMF + q0 + FL],
                w_qkv[:, 2 * DMF + q0 : 2 * DMF + q0 + FL],
            ], axis=1)),
            "bq": np.ascontiguousarray(b_qkv[q0 : q0 + FL]),
            "bk": np.ascontiguousarray(b_qkv[DMF + q0 : DMF + q0 + FL]),
            "bv": np.ascontiguousarray(b_qkv[2 * DMF + q0 : 2 * DMF + q0 + FL]),
            "wo": np.ascontiguousarray(w_out[:, q0 : q0 + FL]),
            "bo": np.ascontiguousarray(b_out[q0 : q0 + FL]),
        })
    return in_maps


def gather_output(results, cfg=None):
    cfg = cfg or Cfg()
    FL = cfg.FLOC
    out = np.empty((BS, cfg.L, cfg.DM), np.float32)
    for c in range(NCORES):
        b, r = divmod(c, GRP)
        out[b, :, r * FL : (r + 1) * FL] = results[c]["out"]
    return out


_PROGRAM = None


def _get_program():
    global _PROGRAM
    if _PROGRAM is None:
        _PROGRAM = make_program()
    return _PROGRAM


def kernel(x, w_qkv, b_qkv, w_out, b_out):
    x = np.asarray(x, np.float32)
    w_qkv = np.asarray(w_qkv, np.float32)
    b_qkv = np.asarray(b_qkv, np.float32)
    w_out = np.asarray(w_out, np.float32)
    b_out = np.asarray(b_out, np.float32)
    nc = _get_program()
    in_maps = shard_inputs(x, w_qkv, b_qkv, w_out, b_out)
    res = run_bass_kernel_spmd(nc, in_maps, list(range(NCORES)))
    return gather_output(res.results)


# revision 35
# speedup vs baseline: 1.0892x; 1.0780x over previous
"""Trainium2 Bass kernel for causal multi-head attention (dense transformer block).

Problem: x[2,2048,1024] -> qkv proj -> 16-head causal attention (scale 1/sqrt(1024))
         -> out proj.  8 NeuronCores.

Sharding: core c handles batch b=c//4 and head-group r=c%4 (heads 4r..4r+3).
  - qkv weights column-sharded by head group (q/k/v slices of 256 cols each)
  - Q/K projection runs in fp8e4 DoubleRow (dm-chunk pairs as the second
    contraction plane, 0.5 cycles/row); Q/K are only consumed by the fp8
    score matmul, so the extra quantization is cheap in accuracy.
  - S^T[k,q] = K^T (stationary) x Q^T (moving), fp8e4 DoubleRow with a
    zeroed second plane (head_dim 64 lives on 64 partitions; plane 1 of K^T
    is zeros so it contracts to 0).  Scores for two k-tiles share one PSUM
    tile so one exp() activation covers both (halves the Act-engine's fixed
    per-instruction SBUF-access overhead).
  - P = exp(S/32) (bf16) with causal masking; attention output accumulated
    token-major: acc[q, 65] += P_blk^T x [V|1] per k-tile (PSUM partition dim
    stays 128; the softmax denominator is a free column, normalized by a
    per-partition reciprocal multiply on the DVE).
  - The out-proj follows an AllGather of O^T (bf16, groups of 4 cores).
  - 4-pass software pipeline: QKV projection for token-block p+1 and deferred
    out-proj work are spliced into pass p's attention job stream so the PE
    stays busy while the Activation engine works through exp().

kernel(**inputs) takes the FULL fp32 inputs and returns the FULL output.
"""

import sys

sys.path.insert(0, "/opt/trn_rl_repo")

import numpy as np

import concourse.bass as bass
import concourse.bacc as bacc
import concourse.mybir as mybir
import concourse.tile as tile
from concourse.bass import ds, ts
from concourse.bass_utils import run_bass_kernel_spmd
from concourse.masks import make_upper_triangular

F32 = mybir.dt.float32
BF16 = mybir.dt.bfloat16
FP8 = mybir.dt.float8e4

# ---------------------------------------------------------------- dims
BS, L, DM, H = 2, 2048, 1024, 16
HD = 64                      # head dim
NCORES = 8
GRP = 4                      # cores per batch group (head-parallel)
HLOC = H // GRP              # heads per core = 4
FLOC = HLOC * HD             # local features = 256
SCALE = 1.0 / float(np.sqrt(DM))
REPLICA_GROUPS = [[0, 1, 2, 3], [4, 5, 6, 7]]


class Cfg:
    def __init__(self, L=L, DM=DM, hloc=HLOC, hd=HD, npass=4, nwarm=32,
                 use_fp8=True, fp8_proj=True):
        self.L, self.DM, self.HLOC, self.HD, self.NPASS = L, DM, hloc, hd, npass
        self.FLOC = hloc * hd
        self.NT = L // 128           # 128-token tiles
        self.NB = L // 512           # 512-token blocks
        self.NDM = DM // 128         # dmodel chunks
        self.PW = L // npass         # pass width (q columns per pass)
        self.NFT = self.FLOC // 128  # feature tiles for O^T (2)
        self.NWARM = nwarm
        self.USE_FP8 = use_fp8
        self.FP8_PROJ = fp8_proj and use_fp8
        self.scale = 1.0 / float(np.sqrt(DM))
        assert self.PW == 512 and self.FLOC % 128 == 0


def build_body(nc, cfg, x, wqkv, bq, bk, bv, wo, bo, out, groups, dbg_hook=None):
    """Emit the per-core program (Tile framework)."""
    NT, NB, NDM, NFT = cfg.NT, cfg.NB, cfg.NDM, cfg.NFT
    HLOCc, HDc, FLOCc = cfg.HLOC, cfg.HD, cfg.FLOC
    Lc, DMc = cfg.L, cfg.DM
    NPASS = cfg.NPASS
    QKDT = FP8 if cfg.USE_FP8 else BF16
    tc = nc.tc
    DR = mybir.MatmulPerfMode.DoubleRow

    with tc.tile_pool(name="const", bufs=1) as constp, \
         tc.tile_pool(name="persist", bufs=1) as pp, \
         tc.tile_pool(name="stage", bufs=2) as sp, \
         tc.tile_pool(name="stage4", bufs=4) as sp4, \
         tc.tile_pool(name="pbuf", bufs=13) as pbp, \
         tc.tile_pool(name="nrm", bufs=4) as nrm, \
         tc.tile_pool(name="otm", bufs=2) as otmp, \
         tc.tile_pool(name="of", bufs=2) as ofp, \
         tc.tile_pool(name="osb", bufs=2) as osbp, \
         tc.tile_pool(name="dram", bufs=2, space="DRAM") as dramp:
        # ---------------- persistent SBUF tensors
        xT = pp.tile([128, NDM, Lc], BF16)                 # x^T  (dm-major)
        wqkvb = pp.tile([128, NDM, 3 * FLOCc], BF16)       # [wq|wk|wv] packed
        wqb = wqkvb[:, :, 0:FLOCc]
        wkb = wqkvb[:, :, FLOCc : 2 * FLOCc]
        wvb = wqkvb[:, :, 2 * FLOCc : 3 * FLOCc]
        wob = pp.tile([128, NDM, FLOCc], BF16)
        if cfg.FP8_PROJ:
            x8T = pp.tile([128, NDM, Lc], FP8)             # fp8 copy of x^T
            wqk8 = pp.tile([128, NDM, 2 * FLOCc], FP8)     # fp8 [wq|wk]
        # Q^T/K^T feature-major with a DoubleRow plane dim: chunk hf holds
        # heads 2hf,2hf+1 (partition 64*(h%2)+hd); plane 0 = data, plane 1 =
        # zeros so the fp8 DoubleRow matmul contracts (K,Q) + (0,0).
        QT8 = pp.tile([128, 2, 2, Lc], QKDT)
        KT8 = pp.tile([128, 2, 2, Lc], QKDT)
        Vb = pp.tile([128, NT, HLOCc * (HDc + 1)], BF16)   # [V | ones] per token tile
        OTs = pp.tile([128, NFT, Lc], BF16)                # attention out^T (feature-major)

        # ---------------- single PSUM pool for the whole kernel
        # banks: stile [128,1024] x2 = 4, acc [128,512] x2 = 2,
        #        work [128,512] x2 = 2  -> 8 banks
        # NOTE: matmul start=True marks the enclosing 2KB zero-region pending,
        # so each accumulation chain owns a full bank (acc padded to 512 f32)
        # and runs its matmuls consecutively.
        psum_cm = tc.tile_pool(name="psum", bufs=2, space="PSUM")
        psum = psum_cm.__enter__()

        # PE warmup: junk matmuls so the p-state ramp happens on the DMA-bound
        # front, not on the first real matmuls.
        wsrc_t = pp.tile([128, 512], BF16, name="wsrc_t")
        nc.vector.memset(wsrc_t, 0.25)
        wps = psum.tile([128, 512], F32, tag="work", name="wps")
        for r in range(cfg.NWARM):
            nc.tensor.matmul(wps, wsrc_t[:, 0:128], wsrc_t,
                             start=(r == 0), stop=(r == cfg.NWARM - 1))
        wout_t = pp.tile([128, 512], F32, name="wout_t")
        nc.vector.tensor_copy(wout_t, wps)
        # preload the Exp activation table during the front
        wexp_t = pp.tile([1, 1], BF16, name="wexp_t")
        nc.scalar.activation(wexp_t, wsrc_t[0:1, 0:1],
                             mybir.ActivationFunctionType.Exp)

        # ---------------- constants
        trimask = constp.tile([128, 128], BF16)
        ones_r = constp.tile([1, 128], BF16)
        bq_f = constp.tile([128, 2], F32)
        bk_f = constp.tile([128, 2], F32)
        bvb = constp.tile([1, FLOCc], BF16)
        bob = constp.tile([1, FLOCc], BF16)

        def emit_consts():
            make_upper_triangular(nc, trimask, val=1.0, diag=True)
            nc.vector.memset(ones_r, 1.0)
            nc.sync.dma_start(bq_f, bq.rearrange("(f p) -> p f", p=128))
            nc.sync.dma_start(bk_f, bk.rearrange("(f p) -> p f", p=128))
            bv_st = constp.tile([1, 2 * FLOCc], F32, name="bv_st")
            nc.sync.dma_start(bv_st[:, 0:FLOCc], bv.rearrange("(a b) -> a b", a=1))
            nc.sync.dma_start(bv_st[:, FLOCc : 2 * FLOCc], bo.rearrange("(a b) -> a b", a=1))
            nc.vector.tensor_copy(bvb, bv_st[:, 0:FLOCc])
            nc.vector.tensor_copy(bob, bv_st[:, FLOCc : 2 * FLOCc])
            nc.vector.memset(
                Vb.rearrange("p t (h u) -> p t h u", u=HDc + 1)[:, :, :, HDc : HDc + 1], 1.0
            )

        def zero_qkplane(b4):
            # zero plane-1 of K^T and Q^T: the DoubleRow second plane must
            # contract to 0, and junk fp8 bytes can decode to NaN (0*NaN=NaN).
            # Split per token block so the front isn't serialized on one big
            # DVE memset.
            nc.vector.memset(KT8[:, :, 1, ts(b4, 512)], 0.0)
            nc.vector.memset(QT8[:, :, 1, ts(b4, 512)], 0.0)

        # ---------------- weight + x staging
        # x rides the single serial SWDGE cast queue; weights go via HWDGE
        # (fp32) + DVE casts, and the fp8 copies are made on the idle Pool
        # engine, so the big casts don't serialize behind each other.
        xv = x.rearrange("(b t p) dm -> b p t dm", p=128, t=4)

        xbfs = {}

        def stage_x_dma(b4):
            xbfs[b4] = sp.tile([128, 4, DMc], BF16, tag="xbf", name="xbf4")
            nc.gpsimd.dma_start(xbfs[b4], xv[b4])

        def stage_x_transpose(b4):
            for t4 in range(4):
                nc.sync.dma_start(
                    xT[:, :, ts(4 * b4 + t4, 128)], xbfs[b4][:, t4, :],
                    transpose=True,
                )
            if cfg.FP8_PROJ:
                # block 0 casts on DVE (front); later blocks on Pool
                if b4 == 0:
                    nc.vector.tensor_copy(x8T[:, :, ts(b4, 512)],
                                          xT[:, :, ts(b4, 512)])
                else:
                    nc.gpsimd.tensor_copy(x8T[:, :, ts(b4, 512)],
                                          xT[:, :, ts(b4, 512)])
            if b4 > 0:
                zero_qkplane(b4)

        # Serial-DMA-conscious order: the SWDGE chain carries only the big
        # cast loads [x0, wqkv, x1, x2, x3, wo]; transposes ride HWDGE as
        # their block lands; fp8 copies are engine-side (DVE for the first
        # block, Pool for the rest).
        stage_x_dma(0)
        nc.gpsimd.dma_start(wqkvb, wqkv.rearrange("(c p) f -> p c f", p=128))
        emit_consts()
        zero_qkplane(0)
        if cfg.FP8_PROJ:
            for c in range(NDM):
                nc.vector.tensor_copy(wqk8[:, c : c + 1, :],
                                      wqkvb[:, c : c + 1, 0 : 2 * FLOCc])
        stage_x_transpose(0)
        for b4 in range(1, NB):
            stage_x_dma(b4)
            stage_x_transpose(b4)
        nc.gpsimd.dma_start(wob, wo.rearrange("(c p) f -> p c f", p=128))

        # ---------------- QKV projection units
        def emit_qk_unit(b, which, ft):
            dst, bias = ((QT8, bq_f) if which == 0 else (KT8, bk_f))
            qk = psum.tile([128, 512], F32, tag="work", name="qk")
            if cfg.FP8_PROJ:
                w8 = wqk8[:, :, which * FLOCc :][:, :, ts(ft, 128)]
                for cc in range(NDM // 2):
                    nc.tensor.matmul(
                        qk, w8[:, 2 * cc : 2 * cc + 2, :],
                        x8T[:, 2 * cc : 2 * cc + 2, ts(b, 512)],
                        start=(cc == 0), stop=(cc == NDM // 2 - 1), perf_mode=DR,
                    )
            else:
                wsl = wqb if which == 0 else wkb
                for c in range(NDM):
                    nc.tensor.matmul(
                        qk, wsl[:, c, ts(ft, 128)], xT[:, c, ts(b, 512)],
                        start=(c == 0), stop=(c == NDM - 1),
                    )
            nc.vector.tensor_scalar_add(dst[:, ft, 0, ts(b, 512)], qk,
                                        bias[:, ft : ft + 1])

        def emit_v_unit(b, sub):
            tt = 4 * b + sub
            psv_full = psum.tile([128, 512], F32, tag="work", name="psv")
            psv = psv_full[:, 0:FLOCc]
            for c in range(NDM):
                nc.tensor.matmul(
                    psv, xT[:, c, ts(tt, 128)], wvb[:, c, :],
                    start=(c == 0), stop=False,
                )
            nc.tensor.matmul(psv, ones_r, bvb, start=False, stop=True)
            nc.vector.tensor_copy(
                Vb[:, tt, :].rearrange("p (h u) -> p h u", u=HDc + 1)[:, :, 0:HDc],
                psv.rearrange("p (h d) -> p h d", d=HDc),
            )

        def qkv_units(b, v_first=False):
            qk = [(lambda w=w, i=i, b=b: emit_qk_unit(b, w, i))
                  for w in (0, 1) for i in (0, 1)]
            v = [(lambda s=s, b=b: emit_v_unit(b, s)) for s in range(4)]
            return v + qk if v_first else qk + v

        for u in qkv_units(0, v_first=True):
            u()

        # ---------------- attention + allgather + out projection
        def emit_scores(p, h, pi):
            """Scores for k-tiles (2*pi, 2*pi+1) in one [128,1024] PSUM tile."""
            hf, hp = h // 2, h % 2
            S = psum.tile([128, 1024], F32, tag="stile", name="S")
            for s in range(2):
                i = 2 * pi + s
                al = max(0, 128 * i - 512 * p)
                if cfg.USE_FP8:
                    nc.tensor.matmul(
                        S[:, ds(512 * s + al, 512 - al)],
                        KT8[64 * hp : 64 * hp + 64, hf, :, ts(i, 128)],
                        QT8[64 * hp : 64 * hp + 64, hf, :, ds(512 * p + al, 512 - al)],
                        start=True, stop=True, perf_mode=DR,
                    )
                else:
                    nc.tensor.matmul(
                        S[:, ds(512 * s + al, 512 - al)],
                        KT8[64 * hp : 64 * hp + 64, hf, 0, ts(i, 128)],
                        QT8[64 * hp : 64 * hp + 64, hf, 0, ds(512 * p + al, 512 - al)],
                        start=True, stop=True,
                    )
            return S

        ag_outs = {}

        def emit_ag(key, q0, qw):
            ag_in = dramp.tile([NFT * 128, qw], BF16, tag=f"agin{qw}", name="ag_in")
            ag_out = dramp.tile([GRP * NFT * 128, qw], BF16, tag=f"agout{qw}",
                                name="ag_out")
            for t in range(NFT):
                nc.sync.dma_start(ag_in[ts(t, 128), :], OTs[:, t, ds(q0, qw)])
            nc.gpsimd.collective_compute(
                "AllGather",
                mybir.AluOpType.bypass,
                ins=[ag_in.opt()],
                outs=[ag_out.opt()],
                replica_groups=groups,
            )
            ag_outs[key] = (ag_out, q0, qw)

        of_tiles = {}

        def emit_of_load(key):
            ag_out, q0, qw = ag_outs[key]
            OF = ofp.tile([128, NDM, 512], BF16, tag="of", name="OF")
            agv = ag_out.rearrange("(c p) q -> c p q", p=128)
            for c in range(NDM):
                nc.sync.dma_start(OF[:, c, 0:qw], agv[c])
            of_tiles[key] = (OF, osbp.tile([128, 4, FLOCc], F32,
                                           tag="osb", name="osb"), q0, qw)

        def emit_op_unit(key, ttl):
            OF, osb, q0, qw = of_tiles[key]
            ntl = qw // 128
            pout_full = psum.tile([128, 512], F32, tag="work", name="pout")
            pout = pout_full[:, 0:FLOCc]
            for c in range(NDM):
                nc.tensor.matmul(
                    pout, OF[:, c, ts(ttl, 128)], wob[:, c, :],
                    start=(c == 0), stop=False,
                )
            nc.tensor.matmul(pout, ones_r, bob, start=False, stop=True)
            nc.vector.tensor_copy(osb[:, ttl, :], pout)
            outv = out[ds(q0, qw), :].rearrange("(t p) f -> p t f", p=128)
            half = ntl // 2
            if ttl == half - 1:
                nc.sync.dma_start(outv[:, 0:half, :], osb[:, 0:half, :])
            if ttl == ntl - 1 and ntl > half:
                nc.sync.dma_start(outv[:, half:ntl, :], osb[:, half:ntl, :])

        def emit_pv_chain(p, h, Ps, j, Otm):
            """One qtile's full accumulation chain (consecutive matmuls into a
            dedicated PSUM bank), then normalize; transpose on the last head."""
            jg = 4 * p + j
            accb = psum.tile([128, 512], F32, tag="acc", name="acc")
            acc = accb[:, 0 : HDc + 1]
            for i in range(jg + 1):
                pi, s = divmod(i, 2)
                nc.tensor.matmul(
                    acc, Ps[pi][:, ds(512 * s + 128 * j, 128)],
                    Vb[:, i, ds((HDc + 1) * h, HDc + 1)],
                    start=(i == 0), stop=(i == jg),
                )
            rec = nrm.tile([128, 1], F32, tag="rec", name="rec")
            nc.vector.reciprocal(rec, accb[:, HDc : HDc + 1])
            nc.vector.tensor_scalar_mul(
                Otm[:, j, ds(HDc * h, HDc)], accb[:, 0:HDc], rec)
            if h == HLOCc - 1:
                nc.sync.dma_start(OTs[:, :, ts(jg, 128)],
                                  Otm[:, j, :], transpose=True)

        for p in range(NPASS):
            npair = 2 * p + 2
            nslot = HLOCc * npair

            units = []
            if p < NPASS - 1:
                units += qkv_units(p + 1)
            if p == 1:
                units.append(lambda: emit_of_load(0))
                units += [(lambda t=t: emit_op_unit(0, t)) for t in range(4)]
            if p == 3:
                units.append(lambda: emit_of_load(1))
                units += [(lambda t=t: emit_op_unit(1, t)) for t in range(4)]
                units.append(lambda: emit_of_load(2))
                units += [(lambda t=t: emit_op_unit(2, t)) for t in range(4)]
            upos = {}
            for k, u in enumerate(units):
                upos.setdefault(1 + (k * (nslot - 2)) // max(1, len(units) - 1),
                                []).append(u)

            Otm = otmp.tile([128, 4, FLOCc], BF16, tag="otm", name="Otm")
            prevPs = None
            slot = 0
            for h in range(HLOCc):
                Ps = []
                for pi in range(npair):
                    S = emit_scores(p, h, pi)
                    al0 = max(0, 128 * (2 * pi) - 512 * p)
                    P = pbp.tile([128, 1024], BF16, tag="ptile", name="P")
                    nc.scalar.activation(
                        P[:, ds(al0, 1024 - al0)], S[:, ds(al0, 1024 - al0)],
                        mybir.ActivationFunctionType.Exp, scale=float(cfg.scale),
                    )
                    for s in range(2):
                        i = 2 * pi + s
                        if i >= 4 * p:  # diagonal block
                            off = 512 * s + 128 * (i - 4 * p)
                            nc.vector.tensor_mul(P[:, ds(off, 128)],
                                                 P[:, ds(off, 128)], trimask)
                    Ps.append(P)
                    for fn in upos.get(slot, ()):
                        fn()
                    if prevPs is not None and pi < 4:
                        emit_pv_chain(p, h - 1, prevPs, pi, Otm)
                    slot += 1
                if prevPs is not None:
                    for j in range(npair, 4):
                        emit_pv_chain(p, h - 1, prevPs, j, Otm)
                prevPs = Ps
            if p < NPASS - 1:
                for j in range(4):
                    emit_pv_chain(p, HLOCc - 1, prevPs, j, Otm)
                emit_ag(p, 512 * p, 512)
            else:
                # last pass: gather + out-project in two halves so the tail
                # after the final normalize is as short as possible
                for j in range(2):
                    emit_pv_chain(p, HLOCc - 1, prevPs, j, Otm)
                emit_ag("3a", 512 * p, 256)
                for j in range(2, 4):
                    emit_pv_chain(p, HLOCc - 1, prevPs, j, Otm)
                emit_ag("3b", 512 * p + 256, 256)
                emit_of_load("3a")
                for t in range(2):
                    emit_op_unit("3a", t)
                emit_of_load("3b")
                for t in range(2):
                    emit_op_unit("3b", t)
        if dbg_hook is not None:
            dbg_hook(locals())
        psum_cm.__exit__(None, None, None)


def make_program(cfg=None, groups=None, unroll=1):
    cfg = cfg or Cfg()
    groups = groups or REPLICA_GROUPS
    nc = bacc.Bacc("TRN2", target_bir_lowering=False, debug=False, num_devices=NCORES)
    x = nc.dram_tensor("x", [cfg.L, cfg.DM], F32, kind="ExternalInput").ap()
    wqkv = nc.dram_tensor("wqkv", [cfg.DM, 3 * cfg.FLOC], F32, kind="ExternalInput").ap()
    bq = nc.dram_tensor("bq", [cfg.FLOC], F32, kind="ExternalInput").ap()
    bk = nc.dram_tensor("bk", [cfg.FLOC], F32, kind="ExternalInput").ap()
    bv = nc.dram_tensor("bv", [cfg.FLOC], F32, kind="ExternalInput").ap()
    wo = nc.dram_tensor("wo", [cfg.DM, cfg.FLOC], F32, kind="ExternalInput").ap()
    bo = nc.dram_tensor("bo", [cfg.FLOC], F32, kind="ExternalInput").ap()
    out = nc.dram_tensor("out", [cfg.L, cfg.FLOC], F32, kind="ExternalOutput").ap()
    with tile.TileContext(nc) as tc:
        nc.tc = tc
        for _ in range(unroll):
            build_body(nc, cfg, x, wqkv, bq, bk, bv, wo, bo, out, groups)
    nc.compile()
    return nc


def shard_inputs(x, w_qkv, b_qkv, w_out, b_out, cfg=None):
    """Full inputs -> list of 8 per-core input dicts."""
    cfg = cfg or Cfg()
    FL = cfg.FLOC
    DMF = cfg.DM
    in_maps = []
    for c in range(NCORES):
        b, r = divmod(c, GRP)
        q0 = r * FL
        in_maps.append({
            "x": np.ascontiguousarray(x[b]),
            "wqkv": np.ascontiguousarray(np.concatenate([
                w_qkv[:, q0 : q0 + FL],
                w_qkv[:, DMF + q0 : DMF + q0 + FL],
                w_qkv[:, 2 * DMF + q0 : 2 * DMF + q0 + FL],
            ], axis=1)),
            "bq": np.ascontiguousarray(b_qkv[q0 : q0 + FL]),
            "bk": np.ascontiguousarray(b_qkv[DMF + q0 : DMF + q0 + FL]),
            "bv": np.ascontiguousarray(b_qkv[2 * DMF + q0 : 2 * DMF + q0 + FL]),
            "wo": np.ascontiguousarray(w_out[:, q0 : q0 + FL]),
            "bo": np.ascontiguousarray(b_out[q0 : q0 + FL]),
        })
    return in_maps


def gather_output(results, cfg=None):
    cfg = cfg or Cfg()
    FL = cfg.FLOC
    out = np.empty((BS, cfg.L, cfg.DM), np.float32)
    for c in range(NCORES):
        b, r = divmod(c, GRP)
        out[b, :, r * FL : (r + 1) * FL] = results[c]["out"]
    return out


_PROGRAM = None


def _get_program():
    global _PROGRAM
    if _PROGRAM is None:
        _PROGRAM = make_program()
    return _PROGRAM


def kernel(x, w_qkv, b_qkv, w_out, b_out):
    x = np.asarray(x, np.float32)
    w_qkv = np.asarray(w_qkv, np.float32)
    b_qkv = np.asarray(b_qkv, np.float32)
    w_out = np.asarray(w_out, np.float32)
    b_out = np.asarray(b_out, np.float32)
    nc = _get_program()
    in_maps = shard_inputs(x, w_qkv, b_qkv, w_out, b_out)
    res = run_bass_kernel_spmd(nc, in_maps, list(range(NCORES)))
    return gather_output(res.results)
